# revision 10
# baseline (speedup 1.0000x reference)
"""Trainium2 Bass kernel for the CMIN video encoder (2x banded MHA + BiGRU).

Self-contained: builds one SPMD Bass program, shards batch across the
8 NeuronCores (8 batches each), runs via run_bass_kernel_spmd, and
reassembles the full [64, 256, 512] output on the host.

Layout strategy: activations kept feature-major ([feature, token]) so every
projection is a plain lhsT=weightT matmul with no transposes. The banded
softmax is computed dense per (batch, head) with exp -> band-mask multiply ->
PE column-sum -> reciprocal scaling. The BiGRU runs both direction chains
interleaved (independent dependency chains pipeline across engines);
W_hh in fp16 (stationary reload each step is the bottleneck; fp16 enables
fast weight load). Sequence-length shifts/reversals and tail zeroing are
data-driven through indirect-DMA row gathers with host-built index tables,
keeping the program identical on all cores.
"""

import os
import numpy as np
import concourse.bass as bass
import concourse.bacc as bacc
import concourse.tile as tile
import concourse.mybir as mybir
from concourse.bass_utils import run_bass_kernel_spmd

B, T, D = 64, 256, 1024
H, DK = 8, D // 8
HID = 512
GH = HID >> 1          # 256
G3 = 3 * GH            # 768
ATTN_WIDTH = 3
NL = 2
NCORES = 8
BC = B // NCORES       # 8 batches per core
NTOK = BC * T          # 2048 token columns per core
SCALE = 1.0 / float(np.sqrt(DK))

F32 = mybir.dt.float32
F32R = mybir.dt.float32r
F16 = mybir.dt.float16
I32 = mybir.dt.int32
AF = mybir.ActivationFunctionType
ALU = mybir.AluOpType

KC = D // 128          # 8 contraction chunks for D
GC = G3 // 128         # 6 gate chunks
HC = GH // 128         # 2 hidden chunks
TT = NTOK // 512       # 4 token tiles of 512
TC = T // 128          # 2 chunks of the T axis
CH = 32                # recurrence steps per gx stream chunk
NCHUNK = T // CH

YROWS = 2 * BC * T + 1  # ystage rows incl. trailing zero row
ZROW = YROWS - 1


def _build(repeat: int = 1, qkv_bias: bool = False, gx_bias: bool = False, phases: str = 'all'):
    nc = bacc.Bacc("TRN2", num_devices=NCORES)

    xT = nc.dram_tensor("xT", [D, NTOK], F32R, kind="ExternalInput")
    wq, wk, wv, wo = [], [], [], []
    for l in range(NL):
        wq.append(nc.dram_tensor(f"WqT{l}", [D, D], F32R, kind="ExternalInput"))
        wk.append(nc.dram_tensor(f"WkT{l}", [D, D], F32R, kind="ExternalInput"))
        wv.append(nc.dram_tensor(f"WvT{l}", [D, D], F32R, kind="ExternalInput"))
        wo.append(nc.dram_tensor(f"WoT{l}", [D, D], F32R, kind="ExternalInput"))
    b_attn = nc.dram_tensor("b_attn", [NL * 4, D], F32, kind="ExternalInput")
    wihf = nc.dram_tensor("WihFT", [D, G3], F32R, kind="ExternalInput")
    wihb = nc.dram_tensor("WihBT", [D, G3], F32R, kind="ExternalInput")
    bih = nc.dram_tensor("bih", [2, G3], F32, kind="ExternalInput")
    whhf = nc.dram_tensor("WhhFT", [GH, G3], F16, kind="ExternalInput")
    whhb = nc.dram_tensor("WhhBT", [GH, G3], F16, kind="ExternalInput")
    band_d = nc.dram_tensor("band", [T, T], F32R, kind="ExternalInput")
    ones_d = nc.dram_tensor("ones", [128, 128], F32R, kind="ExternalInput")
    iden_d = nc.dram_tensor("iden", [128, 128], F32, kind="ExternalInput")
    gxidx_d = nc.dram_tensor("gxidx", [128, NCHUNK * 2], I32, kind="ExternalInput")
    outidx_d = nc.dram_tensor("outidx", [128, BC * 4], I32, kind="ExternalInput")
    yout = nc.dram_tensor("yout", [BC, T, HID], F32, kind="ExternalOutput")

    with (
        nc.allow_low_precision(reason="f32r tiles are fp32-width"),
        tile.TileContext(nc) as tc,
        tc.tile_pool(name="dram", bufs=1, space="DRAM") as dpool,
        tc.tile_pool(name="const", bufs=1) as cpool,
        tc.tile_pool(name="xs", bufs=1) as xpool,
        tc.tile_pool(name="ao", bufs=1) as aopool,
        tc.tile_pool(name="stage", bufs=4) as spool,
        tc.tile_pool(name="psA", bufs=3, space="PSUM") as psA,
    ):
        qf_d = dpool.tile([H, 128, NTOK], F32R, name="qf_d")
        kf_d = dpool.tile([H, 128, NTOK], F32R, name="kf_d")
        vt_d = dpool.tile([NTOK // 128, 128, D], F32R, name="vt_d")
        gx_d = dpool.tile([2 * BC * T, G3], F32, name="gx_d")
        ystage = dpool.tile([YROWS, GH], F32, name="ystage")

        # ---- constants ---------------------------------------------------
        band_t = cpool.tile([128, TC * T], F32R, name="band_t")
        for c in range(TC):
            nc.sync.dma_start(
                band_t[:, c * T:(c + 1) * T], band_d[c * 128:(c + 1) * 128, :]
            )
        ones_t = cpool.tile([128, 128], F32R, name="ones_t")
        nc.sync.dma_start(ones_t[:], ones_d[:])
        iden_t = cpool.tile([128, 128], F32, name="iden_t")
        nc.sync.dma_start(iden_t[:], iden_d[:])
        batt_t = cpool.tile([128, NL * 4 * KC], F32, name="batt_t")
        nc.sync.dma_start(
            batt_t[:], b_attn[:, :].rearrange("r (c p) -> p (r c)", p=128)
        )
        bih_t = cpool.tile([128, 2 * GC], F32, name="bih_t")
        nc.sync.dma_start(bih_t[:], bih[:, :].rearrange("r (c p) -> p (r c)", p=128))
        gxidx_t = cpool.tile([128, NCHUNK * 2], I32, name="gxidx_t")
        nc.sync.dma_start(gxidx_t[:], gxidx_d[:])
        outidx_t = cpool.tile([128, BC * 4], I32, name="outidx_t")
        nc.sync.dma_start(outidx_t[:], outidx_d[:])
        whh_t = cpool.tile([128, 2 * HC * G3], F16, name="whh_t")
        for dr, wd in enumerate((whhf, whhb)):
            for kc in range(HC):
                nc.sync.dma_start(
                    whh_t[:, (dr * HC + kc) * G3:(dr * HC + kc + 1) * G3],
                    wd[kc * 128:(kc + 1) * 128, :],
                )
        zrow_t = cpool.tile([128, HC], F32, name="zrow_t")
        nc.vector.memset(zrow_t[:], 0.0)
        nc.sync.dma_start(
            ystage[ZROW:ZROW + 1, :].rearrange("o (c p) -> p (o c)", p=128),
            zrow_t[:],
        )

        # ---- x resident (feature-major) ---------------------------------
        x_t = xpool.tile([128, KC * NTOK], F32R, name="x_t")
        for kc in range(KC):
            nc.sync.dma_start(
                x_t[:, kc * NTOK:(kc + 1) * NTOK], xT[kc * 128:(kc + 1) * 128, :]
            )

        def xsl(kc, c0=0, n=NTOK):
            return x_t[:, kc * NTOK + c0: kc * NTOK + c0 + n]

        ao_t = aopool.tile([128, H * NTOK], F32R, name="ao_t")

        def load_w_half(wpool, wdram, ncols, h0, hw):
            """Load cols [h0, h0+hw) of a [D, ncols] weight into SBUF; block
            kc occupies wt[:, kc*hw:(kc+1)*hw]."""
            wt = wpool.tile([128, KC * 512], F32R, name="wt", tag="wt")
            for kc in range(KC):
                nc.sync.dma_start(
                    wt[:, kc * hw:(kc + 1) * hw],
                    wdram[kc * 128:(kc + 1) * 128, h0:h0 + hw],
                )
            return wt

        def scope(name):
            sid, _ = nc.enter_named_scope(name)
            return (name, sid)

        def unscope(s):
            nc.leave_named_scope(s[0], s[1])

        def attn_phase(wpool, bhpool, psB):
            for l in range(NL):
                # ============ PASS A: Q, K, V projections -> DRAM ============
                s = scope(f"L{l}_qkv")
                for which, (wdram, outd) in enumerate(((wq[l], qf_d), (wk[l], kf_d))):
                    for half in range(2):
                        wt = load_w_half(wpool, wdram, D, half * 512, 512)
                        for mcl in range(4):
                            mc = half * 4 + mcl
                            for tt in range(TT):
                                ps = psA.tile([128, 512], F32, name="psa", tag="psa")
                                for kc in range(KC):
                                    nc.tensor.matmul(
                                        ps[:],
                                        wt[:, kc * 512 + mcl * 128: kc * 512 + (mcl + 1) * 128],
                                        xsl(kc, tt * 512, 512),
                                        start=(kc == 0),
                                        stop=(kc == KC - 1),
                                    )
                                st = spool.tile([128, 512], F32R, name="st", tag="st")
                                bcol = (l * 4 + which) * KC + mc
                                if qkv_bias:
                                    nc.scalar.activation(
                                        st[:], ps[:], AF.Identity,
                                        bias=batt_t[:, bcol:bcol + 1],
                                    )
                                else:
                                    nc.scalar.activation(st[:], ps[:], AF.Copy)
                                nc.sync.dma_start(
                                    outd[mc, :, tt * 512:(tt + 1) * 512], st[:]
                                )
                # V token-major
                for half in range(2):
                    wt = load_w_half(wpool, wv[l], D, half * 512, 512)
                    for tc_i in range(NTOK // 128):
                        ps = psA.tile([128, 512], F32, name="psv", tag="psa")
                        for kc in range(KC):
                            nc.tensor.matmul(
                                ps[:],
                                xsl(kc, tc_i * 128, 128),
                                wt[:, kc * 512:(kc + 1) * 512],
                                start=(kc == 0),
                                stop=(kc == KC - 1),
                            )
                        st = spool.tile([128, 512], F32R, name="stv", tag="st")
                        nc.vector.tensor_copy(st[:], ps[:])
                        nc.sync.dma_start(
                            vt_d[tc_i, :, half * 512:(half + 1) * 512], st[:]
                        )
                unscope(s)

                # ============ PASS B: banded attention per (b, h) ============
                s = scope(f"L{l}_attn")
                for b in range(BC):
                    for h in range(H):
                        qbh = bhpool.tile([128, T], F32R, name="qbh", tag="qbh")
                        nc.sync.dma_start(qbh[:], qf_d[h, :, b * T:(b + 1) * T])
                        kbh = bhpool.tile([128, T], F32R, name="kbh", tag="kbh")
                        nc.sync.dma_start(kbh[:], kf_d[h, :, b * T:(b + 1) * T])
                        vbh = bhpool.tile([128, T], F32R, name="vbh", tag="vbh")
                        for c in range(TC):
                            nc.sync.dma_start(
                                vbh[:, c * 128:(c + 1) * 128],
                                vt_d[b * TC + c, :, h * 128:(h + 1) * 128],
                            )
                        pm = bhpool.tile([128, TC * T], F32R, name="pm", tag="pm")
                        for c in range(TC):
                            ps = psB.tile([128, T], F32, name="psst", tag="psst")
                            nc.tensor.matmul(
                                ps[:], kbh[:, c * 128:(c + 1) * 128], qbh[:],
                                start=True, stop=True,
                            )
                            pe = bhpool.tile([128, T], F32R, name="pe", tag="pe")
                            nc.scalar.activation(pe[:], ps[:], AF.Exp, scale=SCALE)
                            nc.vector.tensor_mul(
                                pm[:, c * T:(c + 1) * T], pe[:],
                                band_t[:, c * T:(c + 1) * T],
                            )
                        dn = psB.tile([128, T], F32, name="dn", tag="psst")
                        for c in range(TC):
                            nc.tensor.matmul(
                                dn[:], ones_t[:], pm[:, c * T:(c + 1) * T],
                                start=(c == 0), stop=(c == TC - 1),
                            )
                        rr = bhpool.tile([128, T], F32R, name="rr", tag="rr")
                        nc.vector.reciprocal(rr[:], dn[:])
                        for c in range(TC):
                            nc.vector.tensor_mul(
                                pm[:, c * T:(c + 1) * T], pm[:, c * T:(c + 1) * T], rr[:]
                            )
                        av = psB.tile([128, T], F32, name="av", tag="psst")
                        for c in range(TC):
                            nc.tensor.matmul(
                                av[:], vbh[:, c * 128:(c + 1) * 128],
                                pm[:, c * T:(c + 1) * T],
                                start=(c == 0), stop=(c == TC - 1),
                            )
                        nc.scalar.activation(
                            ao_t[:, h * NTOK + b * T: h * NTOK + (b + 1) * T],
                            av[:], AF.Copy,
                        )
                unscope(s)

                # ============ PASS C: O projection + residual (in place) =====
                s = scope(f"L{l}_oproj")
                for half in range(2):
                    wt = load_w_half(wpool, wo[l], D, half * 512, 512)
                    for mcl in range(4):
                        mc = half * 4 + mcl
                        for tt in range(TT):
                            ps = psA.tile([128, 512], F32, name="pso", tag="psa")
                            for kc in range(KC):
                                nc.tensor.matmul(
                                    ps[:],
                                    wt[:, kc * 512 + mcl * 128: kc * 512 + (mcl + 1) * 128],
                                    ao_t[:, kc * NTOK + tt * 512: kc * NTOK + (tt + 1) * 512],
                                    start=(kc == 0),
                                    stop=(kc == KC - 1),
                                )
                            bcol = (l * 4 + 3) * KC + mc
                            nc.vector.scalar_tensor_tensor(
                                xsl(mc, tt * 512, 512),
                                ps[:],
                                batt_t[:, bcol:bcol + 1],
                                xsl(mc, tt * 512, 512),
                                op0=ALU.add,
                                op1=ALU.add,
                            )
                unscope(s)

            # ============ PASS D: GRU input projections -> DRAM ============
            s = scope("gru_proj")
            for dr, wdram in enumerate((wihf, wihb)):
              for half in range(2):
                  wt = load_w_half(wpool, wdram, G3, half * 384, 384)
                  for mcl in range(3):
                      mc = half * 3 + mcl
                      for tt in range(TT):
                          ps = psA.tile([128, 512], F32, name="psg", tag="psa")
                          for kc in range(KC):
                              nc.tensor.matmul(
                                  ps[:],
                                  wt[:, kc * 384 + mcl * 128: kc * 384 + (mcl + 1) * 128],
                                  xsl(kc, tt * 512, 512),
                                  start=(kc == 0),
                                  stop=(kc == KC - 1),
                              )
                          st = spool.tile([128, 512], F32, name="stg", tag="st")
                          bcol = dr * GC + mc
                          if gx_bias:
                              nc.scalar.activation(
                                  st[:], ps[:], AF.Identity,
                                  bias=bih_t[:, bcol:bcol + 1],
                              )
                          else:
                              nc.scalar.activation(st[:], ps[:], AF.Copy)
                          nc.sync.dma_start(
                              gx_d[:, :]
                              .rearrange("(r b t) g -> r b t g", r=2, b=BC)[
                                  dr, tt * 2:(tt + 1) * 2, :, mc * 128:(mc + 1) * 128
                              ]
                              .rearrange("b t g -> g (b t)"),
                              st[:],
                          )
            unscope(s)


        def gru_phase(gxpool, recpool, psR):
            s = scope("gru_rec")
            # ============ PASS E+F: stream gx + run both GRU chains ========
            h_f = recpool.tile([128, HC * BC], F32, name="h_f", tag="hn0", bufs=2)
            h_b = recpool.tile([128, HC * BC], F32, name="h_b", tag="hn1", bufs=2)
            h16 = recpool.tile([128, 2 * HC * BC], F16, name="h16", tag="h16", bufs=2)
            nc.vector.memset(h_f[:], 0.0)
            nc.vector.memset(h_b[:], 0.0)
            nc.vector.memset(h16[:], 0.0)
            for ck in range(NCHUNK):
              gxs = gxpool.tile([128, CH * 96], F32, name="gxs", tag="gxs")
              # fwd: plain strided load (feature-major conversion in the DMA)
              for bb in range(BC):
                for cc in range(GC):
                  nc.sync.dma_start(
                      gxs[:, :]
                      .rearrange("p (j d c b) -> p j d c b", j=CH, d=2, c=GC)[
                          :, :, 0, cc, bb
                      ],
                      gx_d[:, :]
                      .rearrange("(r b t) (c p) -> r b t c p", r=2, b=BC, p=128)[
                          0, bb, ck * CH:(ck + 1) * CH, cc, :
                      ]
                      .rearrange("j p -> p j"),
                  )
              # bwd: indirect row gather in reverse_padded order + PE transpose
              for hf2 in range(2):
                  gb = gxpool.tile([128, G3], F32, name="gb", tag="gb", bufs=2)
                  nc.gpsimd.indirect_dma_start(
                      out=gb[:],
                      out_offset=None,
                      in_=gx_d[:, :],
                      in_offset=bass.IndirectOffsetOnAxis(
                          ap=gxidx_t[:, ck * 2 + hf2: ck * 2 + hf2 + 1], axis=0
                      ),
                  )
                  for c in range(GC):
                      tp = psR.tile([128, 128], F32, name="tp", tag="tp")
                      nc.tensor.transpose(
                          tp[:], gb[:, c * 128:(c + 1) * 128], iden_t[:]
                      )
                      nc.vector.tensor_copy(
                          gxs[:, :]
                          .rearrange("p (j d c b) -> p j d c b", j=CH, d=2, c=GC)[
                              :, :, 1, c, hf2 * 4:(hf2 + 1) * 4
                          ]
                          .rearrange("p j b -> p b j"),
                          tp[:].rearrange("p (b j) -> p b j", b=4),
                      )
              # ---- recurrence steps ----
              for jj in range(CH):
                  j = ck * CH + jj
                  gsl = gxs[:, jj * 96:(jj + 1) * 96]
                  ps_g = psR.tile([128, 96], F32, name="ps_g", tag="ps_g")
                  for dr in range(2):
                      for c in range(GC):
                          for kc in range(HC):
                              nc.tensor.matmul(
                                  ps_g[:, dr * 48 + c * 8: dr * 48 + (c + 1) * 8],
                                  whh_t[:, (dr * HC + kc) * G3 + c * 128:
                                        (dr * HC + kc) * G3 + (c + 1) * 128],
                                  h16[:, (dr * HC + kc) * BC:(dr * HC + kc + 1) * BC],
                                  start=(kc == 0),
                                  stop=(kc == HC - 1),
                              )
                  hnew = []
                  for dr, hcur in enumerate((h_f, h_b)):
                      grz = recpool.tile([128, 32], F32, name="grz", tag=f"grz{dr}")
                      nc.vector.tensor_add(
                          grz[:], ps_g[:, dr * 48: dr * 48 + 32],
                          gsl[:, dr * 48: dr * 48 + 32],
                      )
                      rz = recpool.tile([128, 32], F32, name="rz", tag=f"rz{dr}")
                      nc.scalar.activation(rz[:], grz[:], AF.Sigmoid)
                      t1 = recpool.tile([128, 16], F32, name="t1", tag=f"t1{dr}")
                      nc.vector.tensor_mul(
                          t1[:], rz[:, 0:16], ps_g[:, dr * 48 + 32: dr * 48 + 48]
                      )
                      t2 = recpool.tile([128, 16], F32, name="t2", tag=f"t2{dr}")
                      nc.vector.tensor_add(
                          t2[:], t1[:], gsl[:, dr * 48 + 32: dr * 48 + 48]
                      )
                      n_t = recpool.tile([128, 16], F32, name="n_t", tag=f"n_t{dr}")
                      nc.scalar.activation(n_t[:], t2[:], AF.Tanh)
                      d_t = recpool.tile([128, 16], F32, name="d_t", tag=f"d_t{dr}")
                      nc.gpsimd.tensor_sub(d_t[:], hcur[:], n_t[:])
                      zd = recpool.tile([128, 16], F32, name="zd", tag=f"zd{dr}")
                      nc.vector.tensor_mul(zd[:], rz[:, 16:32], d_t[:])
                      hn = recpool.tile(
                          [128, 16], F32, name="hn", tag=f"hn{dr}", bufs=2
                      )
                      nc.gpsimd.tensor_add(hn[:], n_t[:], zd[:])
                      hnew.append(hn)
                      # y -> staging rows (dr*BC + b)*T + j
                      for cc2 in range(HC):
                          nc.sync.dma_start(
                              ystage[0:2 * BC * T, :]
                              .rearrange("(q t) (c p) -> q t c p", t=T, p=128)[
                                  dr * BC:(dr + 1) * BC, j, cc2, :
                              ]
                              .rearrange("q p -> p q"),
                              hn[:, cc2 * BC:(cc2 + 1) * BC],
                          )
                  h16n = recpool.tile(
                      [128, 2 * HC * BC], F16, name="h16n", tag="h16", bufs=2
                  )
                  nc.vector.tensor_copy(h16n[:, 0:HC * BC], hnew[0][:])
                  nc.vector.tensor_copy(h16n[:, HC * BC:2 * HC * BC], hnew[1][:])
                  h16 = h16n
                  h_f, h_b = hnew
            unscope(s)


        for rep in range(repeat):
            if phases in ("all", "attn"):
                with (
                    tc.tile_pool(name="wt", bufs=2) as wpool,
                    tc.tile_pool(name="bh", bufs=3) as bhpool,
                    tc.tile_pool(name="psB", bufs=2, space="PSUM") as psB,
                ):
                    attn_phase(wpool, bhpool, psB)
            if phases in ("all", "gru"):
                with (
                    tc.tile_pool(name="gx", bufs=2) as gxpool,
                    tc.tile_pool(name="rec", bufs=3) as recpool,
                    tc.tile_pool(name="psR", bufs=2, space="PSUM") as psR,
                ):
                    gru_phase(gxpool, recpool, psR)

            # ============ PASS G: final assembly via row gather ============
            s = scope("assembly")
            for b in range(BC):
              for sc in range(TC):
                  for dr in range(2):
                      col = b * 4 + dr * 2 + sc
                      yt = spool.tile([128, GH], F32, name="yt", tag="yt", bufs=4)
                      nc.gpsimd.indirect_dma_start(
                          out=yt[:],
                          out_offset=None,
                          in_=ystage[:, :],
                          in_offset=bass.IndirectOffsetOnAxis(
                              ap=outidx_t[:, col:col + 1], axis=0
                          ),
                      )
                      nc.sync.dma_start(
                          yout[b, sc * 128:(sc + 1) * 128, dr * GH:(dr + 1) * GH],
                          yt[:],
                      )
            unscope(s)


    nc.compile()
    return nc


_NC_CACHE = {}


def _get_nc(repeat: int = 1):
    if repeat not in _NC_CACHE:
        _NC_CACHE[repeat] = _build(repeat)
    return _NC_CACHE[repeat]


def _host_inputs(inputs, core):
    bs = slice(core * BC, (core + 1) * BC)
    seg = np.asarray(inputs["seg_feats"][bs])
    seglen = np.asarray(inputs["seglen"][bs]).astype(np.int64)

    m = {
        "xT": np.ascontiguousarray(
            seg.transpose(2, 0, 1).reshape(D, NTOK), dtype=np.float32
        )
    }
    for l in range(NL):
        for nm_in, nm_out in (("Wq", "WqT"), ("Wk", "WkT"), ("Wv", "WvT"),
                              ("Wo", "WoT")):
            m[f"{nm_out}{l}"] = np.ascontiguousarray(
                np.asarray(inputs[nm_in][l]).T, dtype=np.float32
            )
    m["b_attn"] = np.stack(
        [np.asarray(inputs[f"b{w}"][l]) for l in range(NL) for w in "qkvo"]
    ).astype(np.float32)
    m["WihFT"] = np.ascontiguousarray(np.asarray(inputs["W_ih_f"]).T, np.float32)
    m["WihBT"] = np.ascontiguousarray(np.asarray(inputs["W_ih_b"]).T, np.float32)
    bhf = np.asarray(inputs["b_hh_f"]).astype(np.float32)
    bhb = np.asarray(inputs["b_hh_b"]).astype(np.float32)
    bif = np.asarray(inputs["b_ih_f"]).astype(np.float32)
    bib = np.asarray(inputs["b_ih_b"]).astype(np.float32)
    # r/z parts of b_hh add inside the same sigmoid as b_ih -> fold them.
    # The n part of b_hh sits inside the r* term; zero in this model.
    assert not np.any(bhf[2 * GH:]) and not np.any(bhb[2 * GH:]), \
        "nonzero b_hh_n not supported"
    m["bih"] = np.stack([
        bif + np.concatenate([bhf[: 2 * GH], np.zeros(GH, np.float32)]),
        bib + np.concatenate([bhb[: 2 * GH], np.zeros(GH, np.float32)]),
    ]).astype(np.float32)
    m["WhhFT"] = np.ascontiguousarray(np.asarray(inputs["W_hh_f"]).T, np.float16)
    m["WhhBT"] = np.ascontiguousarray(np.asarray(inputs["W_hh_b"]).T, np.float16)

    i = np.arange(T)
    m["band"] = (np.abs(i[:, None] - i[None, :]) <= ATTN_WIDTH).astype(np.float32)
    m["ones"] = np.ones((128, 128), np.float32)
    m["iden"] = np.eye(128, dtype=np.float32)

    gxidx = np.zeros((128, NCHUNK * 2), np.int32)
    for ck in range(NCHUNK):
        for hf2 in range(2):
            col = ck * 2 + hf2
            for bl in range(4):
                b = hf2 * 4 + bl
                L = int(seglen[b])
                for jl in range(CH):
                    j = ck * CH + jl
                    src_t = min(max(L - 1 - j, 0), T - 1)
                    gxidx[bl * CH + jl, col] = BC * T + b * T + src_t
    m["gxidx"] = gxidx

    outidx = np.zeros((128, BC * 4), np.int32)
    for b in range(BC):
        L = int(seglen[b])
        for dr in range(2):
            for sc in range(TC):
                col = b * 4 + dr * 2 + sc
                for p in range(128):
                    s = sc * 128 + p
                    if s < L:
                        jrow = s if dr == 0 else L - 1 - s
                        outidx[p, col] = (dr * BC + b) * T + jrow
                    else:
                        outidx[p, col] = ZROW
    m["outidx"] = outidx
    return m


def kernel(**inputs) -> np.ndarray:
    repeat = int(os.environ.get("KERNEL_REPEAT", "1"))
    nc = _get_nc(repeat)
    in_maps = [_host_inputs(inputs, c) for c in range(NCORES)]
    res = run_bass_kernel_spmd(nc, in_maps, core_ids=list(range(NCORES)))
    out = np.concatenate([res.results[c]["yout"] for c in range(NCORES)], axis=0)
    return np.ascontiguousarray(out, dtype=np.float32)



# revision 11
# speedup vs baseline: 22.6707x; 22.6707x over previous
"""Trainium2 Bass kernel for the CMIN video encoder (2x banded MHA + BiGRU).

Self-contained: builds one SPMD Bass program, shards batch across the
8 NeuronCores (8 batches each), runs via run_bass_kernel_spmd, and
reassembles the full [64, 256, 512] output on the host.

Layout strategy: activations kept feature-major ([feature, token]) so every
projection is a plain lhsT=weightT matmul with no transposes. The banded
softmax is computed dense per (batch, head) with exp -> band-mask multiply ->
PE column-sum -> reciprocal scaling. The BiGRU runs both direction chains
interleaved (independent dependency chains pipeline across engines);
W_hh in fp16 (stationary reload each step is the bottleneck; fp16 enables
fast weight load). Sequence-length shifts/reversals and tail zeroing are
data-driven through indirect-DMA row gathers with host-built index tables,
keeping the program identical on all cores.
"""

import os
import numpy as np
import concourse.bass as bass
import concourse.bacc as bacc
import concourse.tile as tile
import concourse.mybir as mybir
from concourse.bass_utils import run_bass_kernel_spmd

B, T, D = 64, 256, 1024
H, DK = 8, D // 8
HID = 512
GH = HID >> 1          # 256
G3 = 3 * GH            # 768
ATTN_WIDTH = 3
NL = 2
NCORES = 8
BC = B // NCORES       # 8 batches per core
NTOK = BC * T          # 2048 token columns per core
SCALE = 1.0 / float(np.sqrt(DK))

F32 = mybir.dt.float32
F32R = mybir.dt.float32r
F16 = mybir.dt.float16
I32 = mybir.dt.int32
AF = mybir.ActivationFunctionType
ALU = mybir.AluOpType

KC = D // 128          # 8 contraction chunks for D
GC = G3 // 128         # 6 gate chunks
HC = GH // 128         # 2 hidden chunks
TT = NTOK // 512       # 4 token tiles of 512
TC = T // 128          # 2 chunks of the T axis
CH = 32                # recurrence steps per gx stream chunk
NCHUNK = T // CH

YROWS = 2 * BC * T + 1  # ystage rows incl. trailing zero row
ZROW = YROWS - 1


def _build(repeat: int = 1, qkv_bias: bool = False, gx_bias: bool = False, phases: str = 'all'):
    nc = bacc.Bacc("TRN2", num_devices=NCORES)

    xT = nc.dram_tensor("xT", [D, NTOK], F32R, kind="ExternalInput")
    wq, wk, wv, wo = [], [], [], []
    for l in range(NL):
        wq.append(nc.dram_tensor(f"WqT{l}", [D, D], F32R, kind="ExternalInput"))
        wk.append(nc.dram_tensor(f"WkT{l}", [D, D], F32R, kind="ExternalInput"))
        wv.append(nc.dram_tensor(f"WvT{l}", [D, D], F32R, kind="ExternalInput"))
        wo.append(nc.dram_tensor(f"WoT{l}", [D, D], F32R, kind="ExternalInput"))
    b_attn = nc.dram_tensor("b_attn", [NL * 4, D], F32, kind="ExternalInput")
    wihf = nc.dram_tensor("WihFT", [D, G3], F32R, kind="ExternalInput")
    wihb = nc.dram_tensor("WihBT", [D, G3], F32R, kind="ExternalInput")
    bih = nc.dram_tensor("bih", [2, G3], F32, kind="ExternalInput")
    whhf = nc.dram_tensor("WhhFT", [GH, G3], F16, kind="ExternalInput")
    whhb = nc.dram_tensor("WhhBT", [GH, G3], F16, kind="ExternalInput")
    band_d = nc.dram_tensor("band", [T, T], F32R, kind="ExternalInput")
    ones_d = nc.dram_tensor("ones", [128, 128], F32R, kind="ExternalInput")
    iden_d = nc.dram_tensor("iden", [128, 128], F32, kind="ExternalInput")
    gxidx_d = nc.dram_tensor("gxidx", [128, NCHUNK * 2], I32, kind="ExternalInput")
    outidx_d = nc.dram_tensor("outidx", [128, BC * 4], I32, kind="ExternalInput")
    yout = nc.dram_tensor("yout", [BC, T, HID], F32, kind="ExternalOutput")

    with (
        nc.allow_low_precision(reason="f32r tiles are fp32-width"),
        tile.TileContext(nc) as tc,
        tc.tile_pool(name="dram", bufs=1, space="DRAM") as dpool,
        tc.tile_pool(name="const", bufs=1) as cpool,
        tc.tile_pool(name="xs", bufs=1) as xpool,
        tc.tile_pool(name="ao", bufs=1) as aopool,
        tc.tile_pool(name="stage", bufs=4) as spool,
        tc.tile_pool(name="psA", bufs=3, space="PSUM") as psA,
    ):
        qf_d = dpool.tile([H, 128, NTOK], F32R, name="qf_d")
        kf_d = dpool.tile([H, 128, NTOK], F32R, name="kf_d")
        vt_d = dpool.tile([NTOK // 128, 128, D], F32R, name="vt_d")
        gx_d = dpool.tile([2 * BC * T, G3], F32, name="gx_d")
        ystage = dpool.tile([YROWS, GH], F32, name="ystage")

        # ---- constants ---------------------------------------------------
        band_t = cpool.tile([128, TC * T], F32R, name="band_t")
        for c in range(TC):
            nc.sync.dma_start(
                band_t[:, c * T:(c + 1) * T], band_d[c * 128:(c + 1) * 128, :]
            )
        ones_t = cpool.tile([128, 128], F32R, name="ones_t")
        nc.sync.dma_start(ones_t[:], ones_d[:])
        iden_t = cpool.tile([128, 128], F32, name="iden_t")
        nc.sync.dma_start(iden_t[:], iden_d[:])
        batt_t = cpool.tile([128, NL * 4 * KC], F32, name="batt_t")
        nc.sync.dma_start(
            batt_t[:], b_attn[:, :].rearrange("r (c p) -> p (r c)", p=128)
        )
        bih_t = cpool.tile([128, 2 * GC], F32, name="bih_t")
        nc.sync.dma_start(bih_t[:], bih[:, :].rearrange("r (c p) -> p (r c)", p=128))
        gxidx_t = cpool.tile([128, NCHUNK * 2], I32, name="gxidx_t")
        nc.sync.dma_start(gxidx_t[:], gxidx_d[:])
        outidx_t = cpool.tile([128, BC * 4], I32, name="outidx_t")
        nc.sync.dma_start(outidx_t[:], outidx_d[:])
        whh_t = cpool.tile([128, 2 * HC * G3], F16, name="whh_t")
        for dr, wd in enumerate((whhf, whhb)):
            for kc in range(HC):
                nc.sync.dma_start(
                    whh_t[:, (dr * HC + kc) * G3:(dr * HC + kc + 1) * G3],
                    wd[kc * 128:(kc + 1) * 128, :],
                )
        zrow_t = cpool.tile([128, HC], F32, name="zrow_t")
        nc.vector.memset(zrow_t[:], 0.0)
        nc.sync.dma_start(
            ystage[ZROW:ZROW + 1, :].rearrange("o (c p) -> p (o c)", p=128),
            zrow_t[:],
        )

        # ---- x resident (feature-major) ---------------------------------
        x_t = xpool.tile([128, KC * NTOK], F32R, name="x_t")
        for kc in range(KC):
            nc.sync.dma_start(
                x_t[:, kc * NTOK:(kc + 1) * NTOK], xT[kc * 128:(kc + 1) * 128, :]
            )

        def xsl(kc, c0=0, n=NTOK):
            return x_t[:, kc * NTOK + c0: kc * NTOK + c0 + n]

        ao_t = aopool.tile([128, H * NTOK], F32R, name="ao_t")

        def load_w_half(wpool, wdram, ncols, h0, hw):
            """Load cols [h0, h0+hw) of a [D, ncols] weight into SBUF; block
            kc occupies wt[:, kc*hw:(kc+1)*hw]."""
            wt = wpool.tile([128, KC * 512], F32R, name="wt", tag="wt")
            for kc in range(KC):
                nc.sync.dma_start(
                    wt[:, kc * hw:(kc + 1) * hw],
                    wdram[kc * 128:(kc + 1) * 128, h0:h0 + hw],
                )
            return wt

        def scope(name):
            sid, _ = nc.enter_named_scope(name, False)
            return (name, sid)

        def unscope(s):
            nc.leave_named_scope(s[0], s[1], False)

        def attn_phase(wpool, bhpool, psB):
            for l in range(NL):
                # ============ PASS A: Q, K, V projections -> DRAM ============
                s = scope(f"L{l}_qkv")
                for which, (wdram, outd) in enumerate(((wq[l], qf_d), (wk[l], kf_d))):
                    for half in range(2):
                        wt = load_w_half(wpool, wdram, D, half * 512, 512)
                        for mcl in range(4):
                            mc = half * 4 + mcl
                            for tt in range(TT):
                                ps = psA.tile([128, 512], F32, name="psa", tag="psa")
                                for kc in range(KC):
                                    nc.tensor.matmul(
                                        ps[:],
                                        wt[:, kc * 512 + mcl * 128: kc * 512 + (mcl + 1) * 128],
                                        xsl(kc, tt * 512, 512),
                                        start=(kc == 0),
                                        stop=(kc == KC - 1),
                                    )
                                st = spool.tile([128, 512], F32R, name="st", tag="st")
                                bcol = (l * 4 + which) * KC + mc
                                if qkv_bias:
                                    nc.scalar.activation(
                                        st[:], ps[:], AF.Identity,
                                        bias=batt_t[:, bcol:bcol + 1],
                                    )
                                else:
                                    nc.scalar.activation(st[:], ps[:], AF.Copy)
                                nc.sync.dma_start(
                                    outd[mc, :, tt * 512:(tt + 1) * 512], st[:]
                                )
                # V token-major
                for half in range(2):
                    wt = load_w_half(wpool, wv[l], D, half * 512, 512)
                    for tc_i in range(NTOK // 128):
                        ps = psA.tile([128, 512], F32, name="psv", tag="psa")
                        for kc in range(KC):
                            nc.tensor.matmul(
                                ps[:],
                                xsl(kc, tc_i * 128, 128),
                                wt[:, kc * 512:(kc + 1) * 512],
                                start=(kc == 0),
                                stop=(kc == KC - 1),
                            )
                        st = spool.tile([128, 512], F32R, name="stv", tag="st")
                        nc.vector.tensor_copy(st[:], ps[:])
                        nc.sync.dma_start(
                            vt_d[tc_i, :, half * 512:(half + 1) * 512], st[:]
                        )
                unscope(s)

                # ============ PASS B: banded attention per (b, h) ============
                s = scope(f"L{l}_attn")
                for b in range(BC):
                    for h in range(H):
                        qbh = bhpool.tile([128, T], F32R, name="qbh", tag="qbh")
                        nc.sync.dma_start(qbh[:], qf_d[h, :, b * T:(b + 1) * T])
                        kbh = bhpool.tile([128, T], F32R, name="kbh", tag="kbh")
                        nc.sync.dma_start(kbh[:], kf_d[h, :, b * T:(b + 1) * T])
                        vbh = bhpool.tile([128, T], F32R, name="vbh", tag="vbh")
                        for c in range(TC):
                            nc.sync.dma_start(
                                vbh[:, c * 128:(c + 1) * 128],
                                vt_d[b * TC + c, :, h * 128:(h + 1) * 128],
                            )
                        pm = bhpool.tile([128, TC * T], F32R, name="pm", tag="pm")
                        for c in range(TC):
                            ps = psB.tile([128, T], F32, name="psst", tag="psst")
                            nc.tensor.matmul(
                                ps[:], kbh[:, c * 128:(c + 1) * 128], qbh[:],
                                start=True, stop=True,
                            )
                            pe = bhpool.tile([128, T], F32R, name="pe", tag="pe")
                            nc.scalar.activation(pe[:], ps[:], AF.Exp, scale=SCALE)
                            nc.vector.tensor_mul(
                                pm[:, c * T:(c + 1) * T], pe[:],
                                band_t[:, c * T:(c + 1) * T],
                            )
                        dn = psB.tile([128, T], F32, name="dn", tag="psst")
                        for c in range(TC):
                            nc.tensor.matmul(
                                dn[:], ones_t[:], pm[:, c * T:(c + 1) * T],
                                start=(c == 0), stop=(c == TC - 1),
                            )
                        rr = bhpool.tile([128, T], F32R, name="rr", tag="rr")
                        nc.vector.reciprocal(rr[:], dn[:])
                        for c in range(TC):
                            nc.vector.tensor_mul(
                                pm[:, c * T:(c + 1) * T], pm[:, c * T:(c + 1) * T], rr[:]
                            )
                        av = psB.tile([128, T], F32, name="av", tag="psst")
                        for c in range(TC):
                            nc.tensor.matmul(
                                av[:], vbh[:, c * 128:(c + 1) * 128],
                                pm[:, c * T:(c + 1) * T],
                                start=(c == 0), stop=(c == TC - 1),
                            )
                        nc.scalar.activation(
                            ao_t[:, h * NTOK + b * T: h * NTOK + (b + 1) * T],
                            av[:], AF.Copy,
                        )
                unscope(s)

                # ============ PASS C: O projection + residual (in place) =====
                s = scope(f"L{l}_oproj")
                for half in range(2):
                    wt = load_w_half(wpool, wo[l], D, half * 512, 512)
                    for mcl in range(4):
                        mc = half * 4 + mcl
                        for tt in range(TT):
                            ps = psA.tile([128, 512], F32, name="pso", tag="psa")
                            for kc in range(KC):
                                nc.tensor.matmul(
                                    ps[:],
                                    wt[:, kc * 512 + mcl * 128: kc * 512 + (mcl + 1) * 128],
                                    ao_t[:, kc * NTOK + tt * 512: kc * NTOK + (tt + 1) * 512],
                                    start=(kc == 0),
                                    stop=(kc == KC - 1),
                                )
                            bcol = (l * 4 + 3) * KC + mc
                            nc.vector.scalar_tensor_tensor(
                                xsl(mc, tt * 512, 512),
                                ps[:],
                                batt_t[:, bcol:bcol + 1],
                                xsl(mc, tt * 512, 512),
                                op0=ALU.add,
                                op1=ALU.add,
                            )
                unscope(s)

            # ============ PASS D: GRU input projections -> DRAM ============
            s = scope("gru_proj")
            for dr, wdram in enumerate((wihf, wihb)):
              for half in range(2):
                  wt = load_w_half(wpool, wdram, G3, half * 384, 384)
                  for mcl in range(3):
                      mc = half * 3 + mcl
                      for tt in range(TT):
                          ps = psA.tile([128, 512], F32, name="psg", tag="psa")
                          for kc in range(KC):
                              nc.tensor.matmul(
                                  ps[:],
                                  wt[:, kc * 384 + mcl * 128: kc * 384 + (mcl + 1) * 128],
                                  xsl(kc, tt * 512, 512),
                                  start=(kc == 0),
                                  stop=(kc == KC - 1),
                              )
                          st = spool.tile([128, 512], F32, name="stg", tag="st")
                          bcol = dr * GC + mc
                          if gx_bias:
                              nc.scalar.activation(
                                  st[:], ps[:], AF.Identity,
                                  bias=bih_t[:, bcol:bcol + 1],
                              )
                          else:
                              nc.scalar.activation(st[:], ps[:], AF.Copy)
                          nc.sync.dma_start(
                              gx_d[:, :]
                              .rearrange("(r b t) g -> r b t g", r=2, b=BC)[
                                  dr, tt * 2:(tt + 1) * 2, :, mc * 128:(mc + 1) * 128
                              ]
                              .rearrange("b t g -> g (b t)"),
                              st[:],
                          )
            unscope(s)


        def gru_phase(gxpool, recpool, psR):
            s = scope("gru_rec")
            # ============ PASS E+F: stream gx + run both GRU chains ========
            h_f = recpool.tile([128, HC * BC], F32, name="h_f", tag="hn0", bufs=2)
            h_b = recpool.tile([128, HC * BC], F32, name="h_b", tag="hn1", bufs=2)
            h16 = recpool.tile([128, 2 * HC * BC], F16, name="h16", tag="h16", bufs=2)
            nc.vector.memset(h_f[:], 0.0)
            nc.vector.memset(h_b[:], 0.0)
            nc.vector.memset(h16[:], 0.0)
            for ck in range(NCHUNK):
              gxs = gxpool.tile([128, CH * 96], F32, name="gxs", tag="gxs")
              # fwd: plain strided load (feature-major conversion in the DMA)
              for bb in range(BC):
                for cc in range(GC):
                  nc.sync.dma_start(
                      gxs[:, :]
                      .rearrange("p (j d c b) -> p j d c b", j=CH, d=2, c=GC)[
                          :, :, 0, cc, bb
                      ],
                      gx_d[:, :]
                      .rearrange("(r b t) (c p) -> r b t c p", r=2, b=BC, p=128)[
                          0, bb, ck * CH:(ck + 1) * CH, cc, :
                      ]
                      .rearrange("j p -> p j"),
                  )
              # bwd: indirect row gather in reverse_padded order + PE transpose
              for hf2 in range(2):
                  gb = gxpool.tile([128, G3], F32, name="gb", tag="gb", bufs=2)
                  nc.gpsimd.indirect_dma_start(
                      out=gb[:],
                      out_offset=None,
                      in_=gx_d[:, :],
                      in_offset=bass.IndirectOffsetOnAxis(
                          ap=gxidx_t[:, ck * 2 + hf2: ck * 2 + hf2 + 1], axis=0
                      ),
                  )
                  for c in range(GC):
                      tp = psR.tile([128, 128], F32, name="tp", tag="tp")
                      nc.tensor.transpose(
                          tp[:], gb[:, c * 128:(c + 1) * 128], iden_t[:]
                      )
                      nc.vector.tensor_copy(
                          gxs[:, :]
                          .rearrange("p (j d c b) -> p j d c b", j=CH, d=2, c=GC)[
                              :, :, 1, c, hf2 * 4:(hf2 + 1) * 4
                          ]
                          .rearrange("p j b -> p b j"),
                          tp[:].rearrange("p (b j) -> p b j", b=4),
                      )
              # ---- recurrence steps ----
              for jj in range(CH):
                  j = ck * CH + jj
                  gsl = gxs[:, jj * 96:(jj + 1) * 96]
                  ps_g = psR.tile([128, 96], F32, name="ps_g", tag="ps_g")
                  for dr in range(2):
                      for c in range(GC):
                          for kc in range(HC):
                              nc.tensor.matmul(
                                  ps_g[:, dr * 48 + c * 8: dr * 48 + (c + 1) * 8],
                                  whh_t[:, (dr * HC + kc) * G3 + c * 128:
                                        (dr * HC + kc) * G3 + (c + 1) * 128],
                                  h16[:, (dr * HC + kc) * BC:(dr * HC + kc + 1) * BC],
                                  start=(kc == 0),
                                  stop=(kc == HC - 1),
                              )
                  hnew = []
                  for dr, hcur in enumerate((h_f, h_b)):
                      grz = recpool.tile([128, 32], F32, name="grz", tag=f"grz{dr}")
                      nc.vector.tensor_add(
                          grz[:], ps_g[:, dr * 48: dr * 48 + 32],
                          gsl[:, dr * 48: dr * 48 + 32],
                      )
                      rz = recpool.tile([128, 32], F32, name="rz", tag=f"rz{dr}")
                      nc.scalar.activation(rz[:], grz[:], AF.Sigmoid)
                      t1 = recpool.tile([128, 16], F32, name="t1", tag=f"t1{dr}")
                      nc.vector.tensor_mul(
                          t1[:], rz[:, 0:16], ps_g[:, dr * 48 + 32: dr * 48 + 48]
                      )
                      t2 = recpool.tile([128, 16], F32, name="t2", tag=f"t2{dr}")
                      nc.vector.tensor_add(
                          t2[:], t1[:], gsl[:, dr * 48 + 32: dr * 48 + 48]
                      )
                      n_t = recpool.tile([128, 16], F32, name="n_t", tag=f"n_t{dr}")
                      nc.scalar.activation(n_t[:], t2[:], AF.Tanh)
                      d_t = recpool.tile([128, 16], F32, name="d_t", tag=f"d_t{dr}")
                      nc.gpsimd.tensor_sub(d_t[:], hcur[:], n_t[:])
                      zd = recpool.tile([128, 16], F32, name="zd", tag=f"zd{dr}")
                      nc.vector.tensor_mul(zd[:], rz[:, 16:32], d_t[:])
                      hn = recpool.tile(
                          [128, 16], F32, name="hn", tag=f"hn{dr}", bufs=2
                      )
                      nc.gpsimd.tensor_add(hn[:], n_t[:], zd[:])
                      hnew.append(hn)
                      # y -> staging rows (dr*BC + b)*T + j
                      for cc2 in range(HC):
                          nc.sync.dma_start(
                              ystage[0:2 * BC * T, :]
                              .rearrange("(q t) (c p) -> q t c p", t=T, p=128)[
                                  dr * BC:(dr + 1) * BC, j, cc2, :
                              ]
                              .rearrange("q p -> p q"),
                              hn[:, cc2 * BC:(cc2 + 1) * BC],
                          )
                  h16n = recpool.tile(
                      [128, 2 * HC * BC], F16, name="h16n", tag="h16", bufs=2
                  )
                  nc.vector.tensor_copy(h16n[:, 0:HC * BC], hnew[0][:])
                  nc.vector.tensor_copy(h16n[:, HC * BC:2 * HC * BC], hnew[1][:])
                  h16 = h16n
                  h_f, h_b = hnew
            unscope(s)


        for rep in range(repeat):
            if phases in ("all", "attn"):
                with (
                    tc.tile_pool(name="wt", bufs=2) as wpool,
                    tc.tile_pool(name="bh", bufs=3) as bhpool,
                    tc.tile_pool(name="psB", bufs=2, space="PSUM") as psB,
                ):
                    attn_phase(wpool, bhpool, psB)
            if phases in ("all", "gru"):
                with (
                    tc.tile_pool(name="gx", bufs=2) as gxpool,
                    tc.tile_pool(name="rec", bufs=3) as recpool,
                    tc.tile_pool(name="psR", bufs=2, space="PSUM") as psR,
                ):
                    gru_phase(gxpool, recpool, psR)

            # ============ PASS G: final assembly via row gather ============
            s = scope("assembly")
            for b in range(BC):
              for sc in range(TC):
                  for dr in range(2):
                      col = b * 4 + dr * 2 + sc
                      yt = spool.tile([128, GH], F32, name="yt", tag="yt", bufs=4)
                      nc.gpsimd.indirect_dma_start(
                          out=yt[:],
                          out_offset=None,
                          in_=ystage[:, :],
                          in_offset=bass.IndirectOffsetOnAxis(
                              ap=outidx_t[:, col:col + 1], axis=0
                          ),
                      )
                      nc.sync.dma_start(
                          yout[b, sc * 128:(sc + 1) * 128, dr * GH:(dr + 1) * GH],
                          yt[:],
                      )
            unscope(s)


    nc.compile()
    return nc


_NC_CACHE = {}


def _get_nc(repeat: int = 1):
    if repeat not in _NC_CACHE:
        _NC_CACHE[repeat] = _build(repeat)
    return _NC_CACHE[repeat]


def _host_inputs(inputs, core):
    bs = slice(core * BC, (core + 1) * BC)
    seg = np.asarray(inputs["seg_feats"][bs])
    seglen = np.asarray(inputs["seglen"][bs]).astype(np.int64)

    m = {
        "xT": np.ascontiguousarray(
            seg.transpose(2, 0, 1).reshape(D, NTOK), dtype=np.float32
        )
    }
    for l in range(NL):
        for nm_in, nm_out in (("Wq", "WqT"), ("Wk", "WkT"), ("Wv", "WvT"),
                              ("Wo", "WoT")):
            m[f"{nm_out}{l}"] = np.ascontiguousarray(
                np.asarray(inputs[nm_in][l]).T, dtype=np.float32
            )
    m["b_attn"] = np.stack(
        [np.asarray(inputs[f"b{w}"][l]) for l in range(NL) for w in "qkvo"]
    ).astype(np.float32)
    m["WihFT"] = np.ascontiguousarray(np.asarray(inputs["W_ih_f"]).T, np.float32)
    m["WihBT"] = np.ascontiguousarray(np.asarray(inputs["W_ih_b"]).T, np.float32)
    bhf = np.asarray(inputs["b_hh_f"]).astype(np.float32)
    bhb = np.asarray(inputs["b_hh_b"]).astype(np.float32)
    bif = np.asarray(inputs["b_ih_f"]).astype(np.float32)
    bib = np.asarray(inputs["b_ih_b"]).astype(np.float32)
    # r/z parts of b_hh add inside the same sigmoid as b_ih -> fold them.
    # The n part of b_hh sits inside the r* term; zero in this model.
    assert not np.any(bhf[2 * GH:]) and not np.any(bhb[2 * GH:]), \
        "nonzero b_hh_n not supported"
    m["bih"] = np.stack([
        bif + np.concatenate([bhf[: 2 * GH], np.zeros(GH, np.float32)]),
        bib + np.concatenate([bhb[: 2 * GH], np.zeros(GH, np.float32)]),
    ]).astype(np.float32)
    m["WhhFT"] = np.ascontiguousarray(np.asarray(inputs["W_hh_f"]).T, np.float16)
    m["WhhBT"] = np.ascontiguousarray(np.asarray(inputs["W_hh_b"]).T, np.float16)

    i = np.arange(T)
    m["band"] = (np.abs(i[:, None] - i[None, :]) <= ATTN_WIDTH).astype(np.float32)
    m["ones"] = np.ones((128, 128), np.float32)
    m["iden"] = np.eye(128, dtype=np.float32)

    gxidx = np.zeros((128, NCHUNK * 2), np.int32)
    for ck in range(NCHUNK):
        for hf2 in range(2):
            col = ck * 2 + hf2
            for bl in range(4):
                b = hf2 * 4 + bl
                L = int(seglen[b])
                for jl in range(CH):
                    j = ck * CH + jl
                    src_t = min(max(L - 1 - j, 0), T - 1)
                    gxidx[bl * CH + jl, col] = BC * T + b * T + src_t
    m["gxidx"] = gxidx

    outidx = np.zeros((128, BC * 4), np.int32)
    for b in range(BC):
        L = int(seglen[b])
        for dr in range(2):
            for sc in range(TC):
                col = b * 4 + dr * 2 + sc
                for p in range(128):
                    s = sc * 128 + p
                    if s < L:
                        jrow = s if dr == 0 else L - 1 - s
                        outidx[p, col] = (dr * BC + b) * T + jrow
                    else:
                        outidx[p, col] = ZROW
    m["outidx"] = outidx
    return m


def kernel(**inputs) -> np.ndarray:
    repeat = int(os.environ.get("KERNEL_REPEAT", "1"))
    nc = _get_nc(repeat)
    in_maps = [_host_inputs(inputs, c) for c in range(NCORES)]
    res = run_bass_kernel_spmd(nc, in_maps, core_ids=list(range(NCORES)))
    out = np.concatenate([res.results[c]["yout"] for c in range(NCORES)], axis=0)
    return np.ascontiguousarray(out, dtype=np.float32)



# revision 13
# speedup vs baseline: 55.0636x; 2.4288x over previous
"""Trainium2 Bass kernel for the CMIN video encoder (2x banded MHA + BiGRU).

v2: the axon execution path charges ~30-150us of wall time PER STATIC
INSTRUCTION (measured; hardware For_i loops execute dynamic iterations at
full speed with no such charge). So this build packs the whole model into
a few hundred static instructions using For_i hardware loops:

- batch loop (8 per core) x per-head loop for both attention layers, with
  weights copied from a resident packed bank into fixed scratch so matmul
  lhsT offsets stay static (ldweights cannot take register offsets).
- banded softmax folded into the score PSUM via an identity-matmul mask add
  (band = 0 / -1e9), then one Exp activation; denominators via ones-matmul.
- BiGRU: forward chain iterates t ascending, backward chain iterates t
  descending over the SAME step program -- pack_padded semantics fall out of
  a +30 bias on the z gate for padded steps (z=1 => carry), so there are no
  reversals and no gathers. h lives in fp16 "ysb" ring tiles [258 blocks of
  (hc,b)] whose block 0/257 stay zero as the initial state for both ends.
- everything stays in SBUF between phases; ~15 DMAs total, all of them
  whole-[128,N] images prepacked on the host.
"""

import os

import ml_dtypes
import numpy as np

import concourse.bacc as bacc
import concourse.mybir as mybir
import concourse.tile as tile
from concourse.bass import ds
from concourse.bass_utils import run_bass_kernel_spmd

B, T, D = 64, 256, 1024
H, DK = 8, D // 8
HID = 512
GH = HID >> 1            # 256
ATTN_WIDTH = 3
NL = 2
NCORES = 8
BC = B // NCORES         # 8 batches per core
KC = D // 128            # 8
OC6 = 6                  # 768 / 128 gate chunks
HC = 2                   # 256 / 128 hidden chunks
TB = T + 2               # ysb time blocks incl zero blocks 0 and 257
SCALE = 1.0 / float(np.sqrt(DK))

F32 = mybir.dt.float32
BF16 = mybir.dt.bfloat16
FP16 = mybir.dt.float16
AF = mybir.ActivationFunctionType


def _build(repeat: int = 1, phases: str = "all"):
    nc = bacc.Bacc("TRN2", num_devices=NCORES)

    x_d = nc.dram_tensor("x_img", [128, KC * BC * T], BF16, kind="ExternalInput")
    wl_d = [
        nc.dram_tensor(f"wl{l}", [128, 4 * 8 * KC * 128], BF16, kind="ExternalInput")
        for l in range(NL)
    ]
    wg_d = nc.dram_tensor("wg", [128, 2 * OC6 * KC * 128], BF16, kind="ExternalInput")
    whh_d = nc.dram_tensor("whh", [128, 2 * OC6 * HC * 128], FP16,
                           kind="ExternalInput")
    iden_d = nc.dram_tensor("iden", [128, 128], BF16, kind="ExternalInput")
    ones_d = nc.dram_tensor("ones", [128, 128], BF16, kind="ExternalInput")
    band_d = nc.dram_tensor("band", [128, 2 * T], BF16, kind="ExternalInput")
    zmask_d = nc.dram_tensor("zmask", [128, OC6 * BC * T], BF16,
                             kind="ExternalInput")
    ymask_d = nc.dram_tensor("ymask", [128, TB * 16], FP16, kind="ExternalInput")
    yout = nc.dram_tensor("yout", [BC, T, HID], FP16, kind="ExternalOutput")

    with (
        nc.allow_low_precision(reason="bf16/fp16 compute within tolerance"),
        tile.TileContext(nc) as tc,
        tc.tile_pool(name="persist", bufs=1) as ppool,
    ):
        def scope(name):
            sid, _ = nc.enter_named_scope(name, False)
            return (name, sid)

        def unscope(s):
            nc.leave_named_scope(s[0], s[1], False)

        # ---- persistent tiles -------------------------------------------
        x_t = ppool.tile([128, KC * BC * T], BF16, name="x_t")
        iden_t = ppool.tile([128, 128], BF16, name="iden_t")
        ones_t = ppool.tile([128, 128], BF16, name="ones_t")
        band_t = ppool.tile([128, 2 * T], BF16, name="band_t")
        ysb_f = ppool.tile([128, TB * 16], FP16, name="ysb_f")
        ysb_b = ppool.tile([128, TB * 16], FP16, name="ysb_b")
        ymask_t = ppool.tile([128, TB * 16], FP16, name="ymask_t")

        nc.sync.dma_start(x_t[:], x_d[:])
        nc.sync.dma_start(iden_t[:], iden_d[:])
        nc.sync.dma_start(ones_t[:], ones_d[:])
        nc.sync.dma_start(band_t[:], band_d[:])
        nc.sync.dma_start(ymask_t[:], ymask_d[:])

        def attn_phase():
            with (
                tc.tile_pool(name="attn", bufs=1) as apool,
                tc.tile_pool(name="psA", bufs=1, space="PSUM") as psAp,
            ):
                wl_t = apool.tile([128, 4 * 8 * KC * 128], BF16, name="wl_t")
                qkv_b = apool.tile([128, 3 * H * T], BF16, name="qkv_b")
                ao_b = apool.tile([128, H * T], BF16, name="ao_b")
                kscr = apool.tile([128, T], BF16, name="kscr")
                vscr = apool.tile([128, T], F32, name="vscr")
                vtscr = apool.tile([128, T], BF16, name="vtscr")
                wcopy = apool.tile([128, 1024], BF16, name="wcopy")
                pm = apool.tile([128, 2 * T], BF16, name="pm")
                rr = apool.tile([128, T], F32, name="rr")
                psA = psAp.tile([128, T], F32, name="psA", tag="psA")
                ps_sc = psAp.tile([128, 2 * T], F32, name="ps_sc", tag="ps_sc")
                ps_dn = psAp.tile([128, T], F32, name="ps_dn", tag="ps_dn")
                ps_av = psAp.tile([128, T], F32, name="ps_av", tag="ps_av")
                ps_vt = psAp.tile([128, 128], F32, name="ps_vt", tag="ps_vt")
                iden32 = apool.tile([128, 128], F32, name="iden32")
                nc.vector.tensor_copy(iden32[:], iden_t[:])

                for l in range(NL):
                    s = scope(f"L{l}")
                    nc.sync.dma_start(wl_t[:], wl_d[l][:])
                    with tc.For_i(0, BC) as bi:
                        # ---- q, k, v projections for batch bi ----
                        for w in range(3):
                            with tc.For_i(0, 8) as oc:
                                nc.vector.tensor_copy(
                                    wcopy[:],
                                    wl_t[:, ds(w * 8192 + oc * 1024, 1024)],
                                )
                                for kc in range(KC):
                                    nc.tensor.matmul(
                                        psA[:],
                                        wcopy[:, kc * 128:(kc + 1) * 128],
                                        x_t[:, ds(kc * BC * T + bi * T, T)],
                                        start=(kc == 0),
                                        stop=(kc == KC - 1),
                                    )
                                nc.scalar.activation(
                                    qkv_b[:, ds(w * H * T + oc * T, T)],
                                    psA[:], AF.Copy,
                                )
                        # ---- banded attention per head ----
                        with tc.For_i(0, H) as h:
                            nc.vector.tensor_copy(
                                kscr[:], qkv_b[:, ds(H * T + h * T, T)]
                            )
                            nc.vector.tensor_copy(
                                vscr[:], qkv_b[:, ds(2 * H * T + h * T, T)]
                            )
                            # vscr is [dk, keys]; av needs keys on partitions
                            for c in range(2):
                                nc.tensor.transpose(
                                    ps_vt[:], vscr[:, c * 128:(c + 1) * 128],
                                    iden32[:],
                                )
                                nc.vector.tensor_copy(
                                    vtscr[:, c * 128:(c + 1) * 128], ps_vt[:]
                                )
                            for c in range(2):
                                nc.tensor.matmul(
                                    ps_sc[:, c * T:(c + 1) * T],
                                    kscr[:, c * 128:(c + 1) * 128],
                                    qkv_b[:, ds(h * T, T)],
                                    start=True, stop=False,
                                )
                                nc.tensor.matmul(
                                    ps_sc[:, c * T:(c + 1) * T],
                                    iden_t[:],
                                    band_t[:, c * T:(c + 1) * T],
                                    start=False, stop=True,
                                )
                            nc.scalar.activation(pm[:], ps_sc[:], AF.Exp,
                                                 scale=SCALE)
                            for c in range(2):
                                nc.tensor.matmul(
                                    ps_dn[:], ones_t[:], pm[:, c * T:(c + 1) * T],
                                    start=(c == 0), stop=(c == 1),
                                )
                            nc.vector.reciprocal(rr[:], ps_dn[:])
                            for c in range(2):
                                nc.tensor.matmul(
                                    ps_av[:], vtscr[:, c * 128:(c + 1) * 128],
                                    pm[:, c * T:(c + 1) * T],
                                    start=(c == 0), stop=(c == 1),
                                )
                            nc.vector.tensor_mul(
                                ao_b[:, ds(h * T, T)], ps_av[:], rr[:]
                            )
                        # ---- output projection + residual ----
                        with tc.For_i(0, 8) as oc2:
                            nc.vector.tensor_copy(
                                wcopy[:],
                                wl_t[:, ds(3 * 8192 + oc2 * 1024, 1024)],
                            )
                            for kc in range(KC):
                                nc.tensor.matmul(
                                    psA[:],
                                    wcopy[:, kc * 128:(kc + 1) * 128],
                                    ao_b[:, kc * T:(kc + 1) * T],
                                    start=(kc == 0),
                                    stop=(kc == KC - 1),
                                )
                            xsl = x_t[:, ds(oc2 * BC * T + bi * T, T)]
                            nc.vector.tensor_add(xsl, psA[:], xsl)
                    unscope(s)

        def gru_phase():
            with (
                tc.tile_pool(name="gru", bufs=1) as gpool,
                tc.tile_pool(name="psG", bufs=1, space="PSUM") as psGp,
            ):
                wg_t = gpool.tile([128, 2 * OC6 * KC * 128], BF16, name="wg_t")
                whh_t = gpool.tile([128, 2 * OC6 * HC * 128], FP16, name="whh_t")
                zmask_t = gpool.tile([128, OC6 * BC * T], BF16, name="zmask_t")
                gx_t = gpool.tile([128, 2 * OC6 * BC * T], BF16, name="gx_t")
                wcopy2 = gpool.tile([128, 1024], BF16, name="wcopy2")
                psA2 = psGp.tile([128, T], F32, name="psA2", tag="psA2")
                ps_g = psGp.tile([128, 96], F32, name="ps_g", tag="ps_g")

                grz = [gpool.tile([128, 32], F32, name=f"grz{d}") for d in range(2)]
                rz = [gpool.tile([128, 32], F32, name=f"rz{d}") for d in range(2)]
                t1 = [gpool.tile([128, 16], F32, name=f"t1{d}") for d in range(2)]
                t2 = [gpool.tile([128, 16], F32, name=f"t2{d}") for d in range(2)]
                nsc = [gpool.tile([128, 16], F32, name=f"n{d}") for d in range(2)]
                dsc = [gpool.tile([128, 16], F32, name=f"d{d}") for d in range(2)]
                zd = [gpool.tile([128, 16], F32, name=f"zd{d}") for d in range(2)]

                s = scope("gru_proj")
                nc.sync.dma_start(wg_t[:], wg_d[:])
                nc.sync.dma_start(whh_t[:], whh_d[:])
                nc.sync.dma_start(zmask_t[:], zmask_d[:])
                nc.vector.memset(ysb_f[:], 0.0)
                nc.vector.memset(ysb_b[:], 0.0)

                # ---- gru input projections: gx = x @ W_ih^T (+ z pad bias)
                with tc.For_i(0, BC) as bi:
                    for dr in range(2):
                        with tc.For_i(0, OC6) as oc:
                            nc.vector.tensor_copy(
                                wcopy2[:],
                                wg_t[:, ds(dr * OC6 * 1024 + oc * 1024, 1024)],
                            )
                            for kc in range(KC):
                                nc.tensor.matmul(
                                    psA2[:],
                                    wcopy2[:, kc * 128:(kc + 1) * 128],
                                    x_t[:, ds(kc * BC * T + bi * T, T)],
                                    start=(kc == 0),
                                    stop=(kc == KC - 1),
                                )
                            nc.vector.tensor_add(
                                gx_t[:, ds(dr * OC6 * BC * T + oc * BC * T
                                           + bi * T, T)],
                                psA2[:],
                                zmask_t[:, ds(oc * BC * T + bi * T, T)],
                            )
                unscope(s)

                # ---- recurrence: fwd ascending t, bwd descending t ----
                s = scope("gru_rec")
                gxv = gx_t[:, :].rearrange(
                    "p (d c b t) -> p d c b t", d=2, c=OC6, b=BC
                )
                ysbv = [
                    y[:, :].rearrange("p (t q) -> p t q", q=16)
                    for y in (ysb_f, ysb_b)
                ]
                with tc.For_i(0, T) as j:
                    for dr, ysb in enumerate((ysb_f, ysb_b)):
                        tt = j if dr == 0 else (T - 1) - j       # time index
                        tr = j if dr == 0 else (T + 1) - j       # read block
                        tw = j + 1 if dr == 0 else T - j         # write block
                        # gate matmuls: gh = W_hh @ h_prev
                        for oc in range(OC6):
                            for kc in range(HC):
                                nc.tensor.matmul(
                                    ps_g[:, dr * 48 + oc * 8:dr * 48 + oc * 8 + 8],
                                    whh_t[:, (dr * OC6 * HC + oc * HC + kc) * 128:
                                          (dr * OC6 * HC + oc * HC + kc + 1) * 128],
                                    ysbv[dr][:, ds(tr, 1), kc * 8:(kc + 1) * 8]
                                    .rearrange("p a q -> p (a q)"),
                                    start=(kc == 0),
                                    stop=(kc == HC - 1),
                                )
                        gx_rz = (
                            gxv[:, dr, 0:4, :, :][:, :, :, ds(tt, 1)]
                            .rearrange("p c b o -> p (c b o)")
                        )
                        gx_n = (
                            gxv[:, dr, 4:6, :, :][:, :, :, ds(tt, 1)]
                            .rearrange("p c b o -> p (c b o)")
                        )
                        nc.vector.tensor_add(
                            grz[dr][:], ps_g[:, dr * 48:dr * 48 + 32], gx_rz
                        )
                        nc.scalar.activation(rz[dr][:], grz[dr][:], AF.Sigmoid)
                        nc.vector.tensor_mul(
                            t1[dr][:], rz[dr][:, 0:16],
                            ps_g[:, dr * 48 + 32:dr * 48 + 48],
                        )
                        nc.vector.tensor_add(t2[dr][:], t1[dr][:], gx_n)
                        nc.scalar.activation(nsc[dr][:], t2[dr][:], AF.Tanh)
                        hprev = (
                            ysbv[dr][:, ds(tr, 1), :].rearrange("p a q -> p (a q)")
                        )
                        nc.vector.tensor_sub(dsc[dr][:], hprev, nsc[dr][:])
                        nc.vector.tensor_mul(zd[dr][:], rz[dr][:, 16:32], dsc[dr][:])
                        nc.vector.tensor_add(
                            ysbv[dr][:, ds(tw, 1), :].rearrange("p a q -> p (a q)"),
                            nsc[dr][:], zd[dr][:],
                        )
                # zero the fwd tail (h carries past L; outputs there must be 0)
                nc.vector.tensor_mul(ysb_f[:], ysb_f[:], ymask_t[:])
                unscope(s)

        for _ in range(repeat):
            if phases in ("all", "attn"):
                attn_phase()
            if phases in ("all", "gru"):
                gru_phase()
            if phases == "attn":
                continue
            # ---- assembly: ysb -> yout (one DMA per direction) ----
            s = scope("assembly")
            for dr, ysb in enumerate((ysb_f, ysb_b)):
                for c in range(HC):
                    for b in range(BC):
                        src = (
                            ysb[:, :]
                            .rearrange("p (t c b) -> p t c b", t=TB, c=HC)[
                                :, 1:T + 1, c, b
                            ]
                        )
                        dst = (
                            yout[b, :, dr * GH + c * 128:
                                 dr * GH + (c + 1) * 128]
                            .rearrange("t p -> p t")
                        )
                        nc.sync.dma_start(dst, src)
            unscope(s)

    nc.compile()
    return nc


_NC_CACHE = {}


def _get_nc(repeat: int = 1):
    if repeat not in _NC_CACHE:
        _NC_CACHE[repeat] = _build(repeat)
    return _NC_CACHE[repeat]


def _pack_w(w, ocn, kcn):
    # [ocn*128, kcn*128] -> [128, ocn*kcn*128] with blocks (oc, kc) of W^T
    a = np.asarray(w).reshape(ocn, 128, kcn, 128)      # (oc, j, kc, p)
    return np.ascontiguousarray(a.transpose(3, 0, 2, 1).reshape(128, -1))


def _host_inputs(inputs, core):
    bs = slice(core * BC, (core + 1) * BC)
    seg = np.asarray(inputs["seg_feats"][bs], dtype=np.float32)
    seglen = np.asarray(inputs["seglen"][bs]).astype(np.int64)

    for nm in ("bq", "bk", "bv", "bo", "b_ih_f", "b_hh_f", "b_ih_b", "b_hh_b"):
        assert not np.any(np.asarray(inputs[nm])), f"nonzero bias {nm} unsupported"

    m = {}
    xt = seg.transpose(2, 0, 1).reshape(KC, 128, BC, T)
    m["x_img"] = np.ascontiguousarray(
        xt.transpose(1, 0, 2, 3).reshape(128, -1)
    ).astype(ml_dtypes.bfloat16)

    for l in range(NL):
        blocks = [
            _pack_w(np.asarray(inputs[nm][l]), 8, 8).astype(ml_dtypes.bfloat16)
            for nm in ("Wq", "Wk", "Wv", "Wo")
        ]
        m[f"wl{l}"] = np.ascontiguousarray(np.concatenate(blocks, axis=1))

    m["wg"] = np.ascontiguousarray(np.concatenate(
        [
            _pack_w(np.asarray(inputs[nm]), OC6, KC).astype(ml_dtypes.bfloat16)
            for nm in ("W_ih_f", "W_ih_b")
        ],
        axis=1,
    ))
    m["whh"] = np.ascontiguousarray(np.concatenate(
        [
            _pack_w(np.asarray(inputs[nm]), OC6, HC).astype(np.float16)
            for nm in ("W_hh_f", "W_hh_b")
        ],
        axis=1,
    ))

    m["iden"] = np.eye(128, dtype=ml_dtypes.bfloat16)
    m["ones"] = np.ones((128, 128), dtype=ml_dtypes.bfloat16)

    p = np.arange(128)[:, None]
    q = np.arange(T)[None, :]
    band = np.concatenate(
        [
            np.where(np.abs((c * 128 + p) - q) <= ATTN_WIDTH, 0.0, -1e9)
            for c in range(2)
        ],
        axis=1,
    ).astype(np.float32)
    m["band"] = band.astype(ml_dtypes.bfloat16)

    t = np.arange(T)[None, :]
    pad = (t >= seglen[:, None]).astype(np.float32)          # [BC, T]
    zrow = np.zeros((OC6, BC, T), np.float32)
    zrow[2] = 30.0 * pad
    zrow[3] = 30.0 * pad
    m["zmask"] = np.broadcast_to(
        zrow.reshape(1, -1), (128, OC6 * BC * T)
    ).astype(ml_dtypes.bfloat16)

    ym = np.zeros((TB, HC, BC), np.float32)
    for b in range(BC):
        ym[1:T + 1, :, b] = (np.arange(T) < seglen[b]).astype(np.float32)[:, None]
    m["ymask"] = np.broadcast_to(
        ym.reshape(1, -1), (128, TB * 16)
    ).astype(np.float16)
    return m


def kernel(**inputs) -> np.ndarray:
    repeat = int(os.environ.get("KERNEL_REPEAT", "1"))
    nc = _get_nc(repeat)
    in_maps = [_host_inputs(inputs, c) for c in range(NCORES)]
    last_err = None
    for _ in range(3):
        try:
            res = run_bass_kernel_spmd(nc, in_maps, core_ids=list(range(NCORES)))
            break
        except Exception as e:  # transient NRT exec failures: retry
            last_err = e
    else:
        raise last_err
    out = np.concatenate(
        [np.asarray(res.results[c]["yout"]) for c in range(NCORES)], axis=0
    )
    return np.ascontiguousarray(out.astype(np.float32))
